# revision 31
# baseline (speedup 1.0000x reference)
"""Trainium2 Bass kernel v3 for nn_Encoder_61830349193463 (retrieval_knn).

Strategy (data-parallel over src rows, 8 NeuronCores):
  - kNN scores via fp16 matmul chain; the -0.5*||a||^2 term is added during
    the PSUM drain on DVE (fp32, mean-shifted) instead of an extra matmul,
    saving 20% of the distance-phase PE streaming.
  - Tile-major phase A: each 128-row tile scans all 8192 anchors, selects
    top-8 per 2048-quarter (DVE max8/max_index), merges to top-5 via an
    iota/is_equal position-match (tie-safe), then gathers rows of a
    host-precomputed table AW2 = anchor_2 @ (W_dim @ W_fus[D:]) / K.
    Gather+mean+transpose pipeline trails one tile behind the matmuls.
  - The W_dim/W_fus algebra folding removes the whole a_map chain: comb is
    assembled as src @ W_fus[:D] + mean(AW2[idx]) + fused bias. The src-half
    fus matmuls are interleaved into phase A per 4-tile block.
  - BatchNorm stats: per-block partial sums (DVE reduce + ACT square-accum),
    AllReduced across cores in halves to hide latency.
  - Output written fp16 feature-major; host transposes + casts to fp32.
"""

import numpy as np

import concourse.bacc as bacc
import concourse.bass as bass
import concourse.mybir as mybir
import concourse.tile as tile
from concourse.bass import IndirectOffsetOnAxis
from concourse.bass_utils import run_bass_kernel_spmd
from concourse.masks import make_identity
import ml_dtypes

F32 = mybir.dt.float32
FP16 = mybir.dt.float16
BF16 = mybir.dt.bfloat16
U32 = mybir.dt.uint32
AF = mybir.ActivationFunctionType
OP = mybir.AluOpType
P = 128

N_FULL, M, D, F = 16384, 8192, 512, 2048
N_CORES = 8
K = 5
EPS = 1e-5


def build_kernel(ns=N_FULL // N_CORES, m=M, d=D, f=F, n_cores=N_CORES):
    DC = d // P          # feature chunks (4)
    FC = f // P          # hidden chunks (16)
    T = ns // P          # n-tiles per core (16)
    nbf = 512            # n-block size for dense chain
    NB = ns // nbf       # n blocks (4)
    MC = m // 512        # 512-wide anchor chunks (16)
    QD = 4               # quarters for top-8 selection
    MQ = m // QD         # anchors per quarter (2048)
    NQ = 8 * QD          # merged candidates per row (32)
    NTOT = float(ns * n_cores)

    nc = bacc.Bacc("TRN2", target_bir_lowering=False, debug=False,
                   num_devices=n_cores)

    def param(name, shape, dt=F32):
        return nc.declare_dram_parameter(name, list(shape), dt, isOutput=False)

    srcT = param("srcT", [d, ns], FP16)
    anchT = param("anchT", [d, m], FP16)
    am2pad = param("am2pad", [P, m], FP16)         # rows 0/1: hi/lo of -0.5||a||^2
    sel2 = param("sel2", [P, P], FP16)             # rows 0/1 ones, rest zero
    aw2 = param("aw2", [m, d], BF16)               # anchor @ (W_dim W_fus_b)/K
    iota32 = param("iota32", [P, NQ], F32)
    rowb = param("rowb", [P, ns // P], F32)        # -est. top score per row
    wfus_r = param("wfus_r", [d, d], FP16)         # src half of W_fus
    we1 = param("we1", [d, f], BF16)
    we2 = param("we2", [f, d], BF16)
    wd = param("wd", [d, d], BF16)
    bfus2 = param("bfus2", [P, DC])                # b_dim @ W_fus_b + b_fus
    be1 = param("be1", [P, FC])
    be2 = param("be2", [P, DC])
    bd = param("bd", [P, DC])
    g1 = param("g1", [P, DC]); bt1 = param("bt1", [P, DC])
    g2 = param("g2", [P, DC]); bt2 = param("bt2", [P, DC])
    gd = param("gd", [P, DC]); btd = param("btd", [P, DC])
    # feature-major fp16 output; host transposes + casts during unshard
    out = nc.declare_dram_parameter("out", [d, ns], FP16, isOutput=True)

    cc_in = [nc.dram_tensor(f"cc{i}_in", [P, 2 * DC], F32) for i in range(6)]
    sync_in = nc.dram_tensor("sync_in", [1, 1], F32)
    cc_space = "Shared" if n_cores > 4 else "Local"
    cc_out = [nc.dram_tensor(f"cc{i}_out", [P, 2 * DC], F32,
                             addr_space=cc_space) for i in range(6)]
    sync_out = nc.dram_tensor("sync_out", [1, 1], F32, addr_space=cc_space)
    groups = [list(range(n_cores))]

    with tile.TileContext(nc) as tc:
        with tc.tile_pool(name="persist", bufs=1) as pp:
            ident_bf = pp.tile([P, P], BF16, name="ident_bf")
            make_identity(nc, ident_bf[:])

            iota_t = pp.tile([P, NQ], F32, name="iota_t")
            nc.sync.dma_start(out=iota_t[:], in_=iota32[:, :])

            sel2_t = pp.tile([P, P], FP16, name="sel2_t")
            nc.sync.dma_start(out=sel2_t[:], in_=sel2[:, :])

            rowb_t = pp.tile([P, T], F32, name="rowb_t")
            nc.sync.dma_start(out=rowb_t[:], in_=rowb[:, :])

            wj_t = pp.tile([P, 512], BF16, name="wj_t")
            nc.vector.memset(wj_t[:], 0.25)

            sTr = []
            for c in range(DC):
                t_ = pp.tile([P, ns], FP16, tag=f"sTr{c}", name=f"sTr{c}")
                nc.sync.dma_start(out=t_[:, :P],
                                  in_=srcT[c * P:(c + 1) * P, :P])
                sTr.append(t_)

            # ACT table prefetch (tanh/square first use otherwise lands on
            # the critical tail)
            scr0 = pp.tile([P, 2], F32, name="scr0")
            nc.vector.memset(scr0[:], 0.0)
            nc.scalar.activation(scr0[:, 0:1], scr0[:, 1:2], AF.Tanh)
            nc.scalar.activation(scr0[:, 0:1], scr0[:, 1:2], AF.Square)

            wp_ctx = tc.tile_pool(name="wpool", bufs=1)
            wp = wp_ctx.__enter__()

            def load_w(t_dram, rows, cols, tag, dt=BF16):
                tiles = []
                for c in range(rows // P):
                    w = wp.tile([P, cols], dt, tag=f"{tag}{c}",
                                name=f"{tag}{c}")
                    nc.sync.dma_start(out=w[:],
                                      in_=t_dram[c * P:(c + 1) * P, :])
                    tiles.append(w)
                return tiles

            wfus_rt = load_w(wfus_r, d, d, "wfusr", dt=FP16)
            wd_t = load_w(wd, d, d, "wd")

            bias_t = {}
            for name, t_dram, cols in [
                    ("bfus2", bfus2, DC), ("be1", be1, FC), ("be2", be2, DC),
                    ("bd", bd, DC), ("g1", g1, DC), ("bt1", bt1, DC),
                    ("g2", g2, DC), ("bt2", bt2, DC), ("gd", gd, DC),
                    ("btd", btd, DC)]:
                bt_ = wp.tile([P, cols], F32, tag=name, name=name)
                nc.sync.dma_start(out=bt_[:], in_=t_dram[:, :])
                bias_t[name] = bt_

            # stats pool + partials (alive through both phases)
            stp_ctx = tc.tile_pool(name="stat", bufs=1)
            stp = stp_ctx.__enter__()
            scr = stp.tile([P, nbf], BF16, tag="scr", name="scr")
            st1p = stp.tile([P, NB * 2 * DC], F32, tag="st1p", name="st1p")
            st2p = stp.tile([P, NB * 2 * DC], F32, tag="st2p", name="st2p")
            st3p = stp.tile([P, NB * 2 * DC], F32, tag="st3p", name="st3p")

            def stat_partial(st_p, tiles, nb, do_sum=True):
                n_sl = slice(nb * nbf, (nb + 1) * nbf)
                for c in range(DC):
                    if do_sum:
                        nc.vector.tensor_reduce(
                            out=st_p[:, nb * 2 * DC + c:nb * 2 * DC + c + 1],
                            in_=tiles[c][:, n_sl], axis=mybir.AxisListType.X,
                            op=OP.add)
                    nc.scalar.activation(
                        scr[:], tiles[c][:, n_sl], AF.Square,
                        accum_out=st_p[:, nb * 2 * DC + DC + c:
                                       nb * 2 * DC + DC + c + 1])

            def stat_push_full(st_p, cidx):
                st = stp.tile([P, 2 * DC], F32, tag=f"st{cidx}",
                              name=f"st{cidx}")
                nc.vector.tensor_reduce(
                    out=st[:],
                    in_=st_p[:, :].rearrange("p (b e) -> p e b", b=NB),
                    axis=mybir.AxisListType.X, op=OP.add)
                nc.sync.dma_start(out=cc_in[cidx][:], in_=st[:])
                nc.gpsimd.collective_compute(
                    "AllReduce", OP.add, replica_groups=groups,
                    ins=[cc_in[cidx].ap()], outs=[cc_out[cidx].ap()])

            def stat_finish_full(cidx):
                gst = stp.tile([P, 2 * DC], F32, tag=f"gstf{cidx}",
                               name=f"gstf{cidx}")
                nc.sync.dma_start(out=gst[:], in_=cc_out[cidx][:])
                mu = stp.tile([P, DC], F32, tag=f"muf{cidx}",
                              name=f"muf{cidx}")
                nc.vector.tensor_scalar(out=mu[:], in0=gst[:, :DC],
                                        scalar1=1.0 / NTOT, scalar2=None,
                                        op0=OP.mult)
                musq = stp.tile([P, DC], F32, tag=f"musqf{cidx}",
                                name=f"musqf{cidx}")
                nc.vector.tensor_tensor(out=musq[:], in0=mu[:], in1=mu[:],
                                        op=OP.mult)
                var = stp.tile([P, DC], F32, tag=f"varf{cidx}",
                               name=f"varf{cidx}")
                nc.vector.scalar_tensor_tensor(
                    out=var[:], in0=gst[:, DC:], scalar=1.0 / NTOT,
                    in1=musq[:], op0=OP.mult, op1=OP.subtract)
                sd = stp.tile([P, DC], F32, tag=f"sdf{cidx}",
                              name=f"sdf{cidx}")
                nc.vector.tensor_scalar(out=sd[:], in0=var[:], scalar1=EPS,
                                        scalar2=None, op0=OP.add)
                nc.scalar.sqrt(sd[:], sd[:])
                rs = stp.tile([P, DC], F32, tag=f"rsf{cidx}",
                              name=f"rsf{cidx}")
                nc.vector.reciprocal(rs[:], sd[:])
                return mu, rs

            def stat_push(st_p, idx, half):
                cidx = 2 * idx + half
                st = stp.tile([P, 2 * DC], F32, tag=f"st{cidx}",
                              name=f"st{cidx}")
                nc.vector.tensor_reduce(
                    out=st[:],
                    in_=st_p[:, half * 4 * DC:(half + 1) * 4 * DC]
                    .rearrange("p (b e) -> p e b", b=NB // 2),
                    axis=mybir.AxisListType.X, op=OP.add)
                nc.sync.dma_start(out=cc_in[cidx][:], in_=st[:])
                nc.gpsimd.collective_compute(
                    "AllReduce", OP.add, replica_groups=groups,
                    ins=[cc_in[cidx].ap()], outs=[cc_out[cidx].ap()])

            def stat_finish(idx):
                ga = stp.tile([P, 2 * DC], F32, tag=f"ga{idx}",
                              name=f"ga{idx}")
                nc.sync.dma_start(out=ga[:], in_=cc_out[2 * idx][:])
                gb = stp.tile([P, 2 * DC], F32, tag=f"gb{idx}",
                              name=f"gb{idx}")
                nc.sync.dma_start(out=gb[:], in_=cc_out[2 * idx + 1][:])
                gst = stp.tile([P, 2 * DC], F32, tag=f"gst{idx}",
                               name=f"gst{idx}")
                nc.vector.tensor_tensor(out=gst[:], in0=ga[:], in1=gb[:],
                                        op=OP.add)
                mu = stp.tile([P, DC], F32, tag=f"mu{idx}", name=f"mu{idx}")
                nc.vector.tensor_scalar(out=mu[:], in0=gst[:, :DC],
                                        scalar1=1.0 / NTOT, scalar2=None,
                                        op0=OP.mult)
                musq = stp.tile([P, DC], F32, tag=f"musq{idx}",
                                name=f"musq{idx}")
                nc.vector.tensor_tensor(out=musq[:], in0=mu[:], in1=mu[:],
                                        op=OP.mult)
                var = stp.tile([P, DC], F32, tag=f"var{idx}",
                               name=f"var{idx}")
                nc.vector.scalar_tensor_tensor(
                    out=var[:], in0=gst[:, DC:], scalar=1.0 / NTOT,
                    in1=musq[:], op0=OP.mult, op1=OP.subtract)
                sd = stp.tile([P, DC], F32, tag=f"sd{idx}", name=f"sd{idx}")
                nc.vector.tensor_scalar(out=sd[:], in0=var[:], scalar1=EPS,
                                        scalar2=None, op0=OP.add)
                nc.scalar.sqrt(sd[:], sd[:])
                rs = stp.tile([P, DC], F32, tag=f"rs{idx}", name=f"rs{idx}")
                nc.vector.reciprocal(rs[:], sd[:])
                return mu, rs

            def bn_affine(mu, rs, gname, bname, idx):
                s = stp.tile([P, DC], F32, tag=f"s{idx}", name=f"s{idx}")
                nc.vector.tensor_tensor(out=s[:], in0=rs[:],
                                        in1=bias_t[gname][:], op=OP.mult)
                tmp = stp.tile([P, DC], F32, tag=f"tmp{idx}",
                               name=f"tmp{idx}")
                nc.vector.tensor_tensor(out=tmp[:], in0=mu[:], in1=s[:],
                                        op=OP.mult)
                tb = stp.tile([P, DC], F32, tag=f"tb{idx}", name=f"tb{idx}")
                nc.vector.tensor_tensor(out=tb[:], in0=bias_t[bname][:],
                                        in1=tmp[:], op=OP.subtract)
                return s, tb

            combraw = [stp.tile([P, ns], BF16, tag=f"craw{c}",
                                name=f"craw{c}") for c in range(DC)]

            # ================= PHASE A + interleaved fus =================
            with (
                tc.tile_pool(name="anch", bufs=1) as ap_anch,
                tc.tile_pool(name="simp", bufs=2) as sim_pool,
                tc.tile_pool(name="dps", bufs=4, space="PSUM") as dps,
                tc.tile_pool(name="tps", bufs=2, space="PSUM") as tpsp,
                tc.tile_pool(name="fps", bufs=2, space="PSUM") as fps,
                tc.tile_pool(name="cand", bufs=2) as cand_pool,
                tc.tile_pool(name="mrg", bufs=2) as mrg,
                tc.tile_pool(name="gat", bufs=2) as gat,
            ):
                am2_t = ap_anch.tile([P, m], FP16, name="am2_t")
                nc.sync.dma_start(out=am2_t[:, :MQ], in_=am2pad[:, :MQ])
                aqr = [ap_anch.tile([P, m], FP16, tag=f"aqr{c}",
                                    name=f"aqr{c}") for c in range(DC)]
                for mc in range(MC):
                    msl = slice(mc * 512, (mc + 1) * 512)
                    for c in range(DC):
                        nc.sync.dma_start(out=aqr[c][:, msl],
                                          in_=anchT[c * P:(c + 1) * P, msl])
                    if mc == 3:
                        nc.sync.dma_start(out=am2_t[:, MQ:],
                                          in_=am2pad[:, MQ:])
                        for c in range(DC):
                            nc.sync.dma_start(
                                out=sTr[c][:, P:],
                                in_=srcT[c * P:(c + 1) * P, P:])

                Gs = [None] * T

                def tile_distance(t):
                    n_sl = slice(t * P, (t + 1) * P)
                    vcand = cand_pool.tile([P, NQ], FP16, tag="vc", name="vc")
                    icand = cand_pool.tile([P, NQ], F32, tag="ic", name="ic")
                    for q in range(QD):
                        simq = sim_pool.tile([P, MQ], FP16, tag="simq",
                                             name="simq")
                        for sc in range(MQ // 512):
                            mc = q * (MQ // 512) + sc
                            msl = slice(mc * 512, (mc + 1) * 512)
                            ssl = slice(sc * 512, (sc + 1) * 512)
                            ps = dps.tile([P, 512], F32, name="dps")
                            for c in range(DC):
                                nc.tensor.matmul(ps[:], sTr[c][:, n_sl],
                                                 aqr[c][:, msl],
                                                 start=(c == 0), stop=False)
                            nc.tensor.matmul(ps[:], sel2_t[:],
                                             am2_t[:, msl],
                                             start=False, stop=True)
                            nc.scalar.activation(simq[:, ssl], ps[:],
                                                 AF.Identity,
                                                 bias=rowb_t[:, t:t + 1])
                        nc.vector.max(out=vcand[:, q * 8:(q + 1) * 8],
                                      in_=simq[:])
                        i8 = mrg.tile([P, 8], U32, tag="i8", name="i8")
                        nc.vector.max_index(out=i8[:],
                                            in_max=vcand[:, q * 8:(q + 1) * 8],
                                            in_values=simq[:])
                        nc.vector.tensor_scalar(
                            out=icand[:, q * 8:(q + 1) * 8], in0=i8[:],
                            scalar1=float(q * MQ), scalar2=None, op0=OP.add)
                        if q == 1 and t >= 2:
                            tile_mean(t - 2)
                    # merge: top-8 values of the 32 candidates, positions via
                    # max_index, then position->anchor-id via iota/is_equal
                    g8 = mrg.tile([P, 8], FP16, tag="g8", name="g8")
                    nc.vector.max(out=g8[:], in_=vcand[:])
                    p8 = mrg.tile([P, 8], U32, tag="p8", name="p8")
                    nc.vector.max_index(out=p8[:], in_max=g8[:],
                                        in_values=vcand[:])
                    p8f = mrg.tile([P, 8], F32, tag="p8f", name="p8f")
                    nc.vector.tensor_copy(p8f[:], p8[:])
                    eqm = mrg.tile([P, 8 * NQ], FP16, tag="eqm", name="eqm")
                    nc.vector.tensor_tensor(
                        out=eqm[:].rearrange("p (a b) -> p a b", a=8),
                        in0=iota_t[:].rearrange(
                            "p (a q) -> p a q", a=1).to_broadcast([P, 8, NQ]),
                        in1=p8f[:].rearrange(
                            "p (a o) -> p a o", o=1).to_broadcast([P, 8, NQ]),
                        op=OP.is_equal)
                    prod = mrg.tile([P, 8 * NQ], F32, tag="prod", name="prod")
                    nc.vector.tensor_tensor(
                        out=prod[:].rearrange("p (a b) -> p a b", a=8),
                        in0=eqm[:].rearrange("p (a b) -> p a b", a=8),
                        in1=icand[:].rearrange(
                            "p (a q) -> p a q", a=1).to_broadcast([P, 8, NQ]),
                        op=OP.mult)
                    idx8f = mrg.tile([P, 8], F32, tag="idx8f", name="idx8f")
                    nc.vector.tensor_reduce(
                        out=idx8f[:],
                        in_=prod[:].rearrange("p (a b) -> p a b", a=8),
                        axis=mybir.AxisListType.X, op=OP.add)
                    idx8 = mrg.tile([P, 8], U32, tag="idx8", name="idx8")
                    nc.vector.tensor_copy(idx8[:], idx8f[:])
                    G = gat.tile([P, K * d], BF16, tag="G", name="G")
                    for k in range(K):
                        nc.gpsimd.indirect_dma_start(
                            out=G[:, k * d:(k + 1) * d], out_offset=None,
                            in_=aw2[:],
                            in_offset=IndirectOffsetOnAxis(
                                ap=idx8[:, k:k + 1], axis=0))
                    Gs[t] = G

                mrs = [None] * T

                def tile_mean(t):
                    # DVE reduce only; emitted mid-tile t+1 so the transpose
                    # at the top of tile t+2 never waits on it
                    mr = mnr.tile([P, d], BF16, tag="mr", name="mr")
                    with nc.allow_low_precision(
                            reason="sum of 5 bf16 values; |x|~0.05"):
                        nc.vector.tensor_reduce(
                            out=mr[:],
                            in_=Gs[t][:].rearrange("p (k e) -> p e k", k=K),
                            axis=mybir.AxisListType.X, op=OP.add)
                    mrs[t] = mr

                def tile_transpose(t):
                    # combraw[c][:, t*128 + n] = sum_k G[n, k*d + c*128+dp]
                    # (fus matmul result is added in place later)
                    mr = mrs[t]
                    tp = tpsp.tile([P, d], F32, name="tp")
                    for j in range(DC):
                        nc.tensor.matmul(tp[:, j * P:(j + 1) * P],
                                         mr[:, j * P:(j + 1) * P],
                                         ident_bf[:], start=True, stop=True)
                    for c in range(DC):
                        nc.scalar.copy(combraw[c][:, t * P:(t + 1) * P],
                                       tp[:, c * P:(c + 1) * P])

                def pe_keepwarm(n_mm, pool, tag):
                    wps = pool.tile([P, nbf], F32, tag=tag, name=tag)
                    for i in range(n_mm):
                        nc.tensor.matmul(wps[:], ident_bf[:], wj_t[:],
                                         start=(i == 0), stop=(i == n_mm - 1))
                    nc.scalar.copy(scr0[:, 0:1], wps[:, 0:1])

                def fus_block(nb):
                    n_sl = slice(nb * nbf, (nb + 1) * nbf)
                    for fc in range(DC):
                        ps = fps.tile([P, nbf], F32, name="fps")
                        for c in range(DC):
                            nc.tensor.matmul(
                                ps[:], wfus_rt[c][:, fc * P:(fc + 1) * P],
                                sTr[c][:, n_sl],
                                start=(c == 0), stop=(c == DC - 1))
                        nc.vector.scalar_tensor_tensor(
                            out=combraw[fc][:, n_sl], in0=ps[:],
                            scalar=bias_t["bfus2"][:, fc:fc + 1],
                            in1=combraw[fc][:, n_sl],
                            op0=OP.add, op1=OP.add)
                    stat_partial(st1p, combraw, nb, do_sum=True)

                for t in range(T):
                    if t >= 3:
                        tile_transpose(t - 3)
                    if t >= 6 and (t - 6) % 4 == 0:
                        fus_block((t - 6) // 4)
                        if (t - 6) // 4 == 1:
                            stat_push(st1p, 0, 0)
                    if t == 12:
                        nc.gpsimd.collective_compute(
                            "AllReduce", OP.add, replica_groups=groups,
                            ins=[sync_in.ap()], outs=[sync_out.ap()])
                    tile_distance(t)
                tile_mean(T - 2)
                tile_transpose(T - 3)
                tile_transpose(T - 2)
                tile_mean(T - 1)
                tile_transpose(T - 1)
                fus_block(NB - 1)
                stat_push(st1p, 0, 1)
                pe_keepwarm(40, fps, "fps")

            # we1/we2 load into the freed anchor space during the BN1
            # AllReduce + start of the MLP
            we1_t = load_w(we1, d, f, "we1")
            we2_t = load_w(we2, f, d, "we2")

            # ================= PHASE B: dense chain =================
            with (
                tc.tile_pool(name="act", bufs=1) as ap_,
                tc.tile_pool(name="mlp", bufs=1) as mp_,
                tc.tile_pool(name="bps", bufs=4, space="PSUM") as bps,
                tc.tile_pool(name="onat", bufs=2) as onp,
            ):
                mu1, rs1 = stat_finish(0)
                s1, t1 = bn_affine(mu1, rs1, "g1", "bt1", 0)
                combT = [ap_.tile([P, ns], BF16, tag=f"combT{c}",
                                  name=f"combT{c}") for c in range(DC)]
                r2T = [ap_.tile([P, ns], BF16, tag=f"r2T{c}", name=f"r2T{c}")
                       for c in range(DC)]
                for nb in range(NB):
                    n_sl = slice(nb * nbf, (nb + 1) * nbf)
                    for c in range(DC):
                        nc.scalar.activation(combT[c][:, n_sl],
                                             combraw[c][:, n_sl],
                                             AF.Identity, bias=t1[:, c:c + 1],
                                             scale=s1[:, c:c + 1])
                    tT = [mp_.tile([P, nbf], BF16, tag=f"tT{fe}",
                                   name=f"tT{fe}") for fe in range(FC)]
                    for fe in range(FC):
                        ps = bps.tile([P, nbf], F32, tag="psB", name="psB")
                        for c in range(DC):
                            nc.tensor.matmul(
                                ps[:], we1_t[c][:, fe * P:(fe + 1) * P],
                                combT[c][:, n_sl],
                                start=(c == 0), stop=(c == DC - 1))
                        nc.scalar.activation(tT[fe][:], ps[:], AF.Tanh,
                                             bias=bias_t["be1"][:, fe:fe + 1])
                    for fc in range(DC):
                        ps = bps.tile([P, nbf], F32, tag="psB", name="psB")
                        for fe in range(FC):
                            nc.tensor.matmul(
                                ps[:], we2_t[fe][:, fc * P:(fc + 1) * P],
                                tT[fe][:],
                                start=(fe == 0), stop=(fe == FC - 1))
                        nc.vector.scalar_tensor_tensor(
                            out=r2T[fc][:, n_sl], in0=ps[:],
                            scalar=bias_t["be2"][:, fc:fc + 1],
                            in1=combT[fc][:, n_sl], op0=OP.add, op1=OP.add)
                    stat_partial(st2p, r2T, nb, do_sum=True)
                    if nb == 1:
                        stat_push(st2p, 1, 0)
                stat_push(st2p, 1, 1)
                pe_keepwarm(60, bps, "psB")

                mu2, rs2 = stat_finish(1)
                s2, t2 = bn_affine(mu2, rs2, "g2", "bt2", 1)
                c2T = combraw  # reuse buffers
                yT = [ap_.tile([P, ns], BF16, tag=f"yT{c}", name=f"yT{c}")
                      for c in range(DC)]
                for nb in range(NB):
                    n_sl = slice(nb * nbf, (nb + 1) * nbf)
                    for c in range(DC):
                        nc.scalar.activation(c2T[c][:, n_sl],
                                             r2T[c][:, n_sl], AF.Identity,
                                             bias=t2[:, c:c + 1],
                                             scale=s2[:, c:c + 1])
                    for fc in range(DC):
                        ps = bps.tile([P, nbf], F32, tag="psB", name="psB")
                        for c in range(DC):
                            nc.tensor.matmul(
                                ps[:], wd_t[c][:, fc * P:(fc + 1) * P],
                                c2T[c][:, n_sl],
                                start=(c == 0), stop=(c == DC - 1))
                        nc.scalar.activation(
                            yT[fc][:, n_sl], ps[:], AF.Identity,
                            bias=bias_t["bd"][:, fc:fc + 1],
                            accum_out=st3p[:, nb * 2 * DC + fc:
                                           nb * 2 * DC + fc + 1])
                    stat_partial(st3p, yT, nb, do_sum=False)
                    if nb == 1:
                        stat_push(st3p, 2, 0)
                stat_push(st3p, 2, 1)
                pe_keepwarm(60, bps, "psB")

                mu3, rs3 = stat_finish(2)
                s3, t3 = bn_affine(mu3, rs3, "gd", "btd", 2)
                for j in range(DC):
                    oj = onp.tile([P, ns], FP16, tag="oj", name="oj")
                    nc.scalar.activation(oj[:], yT[j][:], AF.Tanh,
                                         bias=t3[:, j:j + 1],
                                         scale=s3[:, j:j + 1])
                    nc.sync.dma_start(out=out[j * P:(j + 1) * P, :],
                                      in_=oj[:])

            stp_ctx.__exit__(None, None, None)
            wp_ctx.__exit__(None, None, None)

    nc.finalize()
    return nc


def _chunk_vec(v, cols):
    return np.ascontiguousarray(v.reshape(cols, P).T)


def prepare_inputs(src, anchor_2, W_dim, b_dim, W_fus, b_fus, W_e1, b_e1,
                   W_e2, b_e2, g1, bt1, g2, bt2, W_d, b_d, g_d, bt_d,
                   n_cores=N_CORES, ns=N_FULL // N_CORES):
    d = src.shape[1]
    f = W_e1.shape[1]
    DC, FC = d // P, f // P
    src = np.asarray(src, np.float32)
    anchor_2 = np.asarray(anchor_2, np.float32)
    wfus_b = np.asarray(W_fus, np.float32)[d:]
    Wc = (np.asarray(W_dim, np.float32) @ wfus_b) / K
    aw2 = (anchor_2 @ Wc).astype(ml_dtypes.bfloat16)
    bfus2 = np.asarray(b_dim, np.float32) @ wfus_b + np.asarray(b_fus,
                                                               np.float32)
    am2 = -0.5 * (anchor_2.astype(np.float64) ** 2).sum(1)
    am2 -= am2.mean()
    am2_h = np.float16(am2)
    am2_l = np.float16(am2 - am2_h.astype(np.float64))
    am2pad = np.zeros((P, am2.shape[0]), np.float16)
    am2pad[0] = am2_h
    am2pad[1] = am2_l
    sel2 = np.zeros((P, P), np.float16)
    sel2[0:2, :] = 1.0
    iota32 = np.ascontiguousarray(
        np.broadcast_to(np.arange(32, dtype=np.float32), (P, 32)))
    shared = dict(
        anchT=np.ascontiguousarray(anchor_2.T).astype(np.float16),
        am2pad=am2pad,
        sel2=sel2,
        aw2=aw2,
        iota32=iota32,
        wfus_r=np.ascontiguousarray(
            np.asarray(W_fus, np.float32)[:d]).astype(np.float16),
        we1=np.asarray(W_e1).astype(ml_dtypes.bfloat16),
        we2=np.asarray(W_e2).astype(ml_dtypes.bfloat16),
        wd=np.asarray(W_d).astype(ml_dtypes.bfloat16),
        bfus2=_chunk_vec(bfus2.astype(np.float32), DC),
        be1=_chunk_vec(np.asarray(b_e1, np.float32), FC),
        be2=_chunk_vec(np.asarray(b_e2, np.float32), DC),
        bd=_chunk_vec(np.asarray(b_d, np.float32), DC),
        g1=_chunk_vec(np.asarray(g1, np.float32), DC),
        bt1=_chunk_vec(np.asarray(bt1, np.float32), DC),
        g2=_chunk_vec(np.asarray(g2, np.float32), DC),
        bt2=_chunk_vec(np.asarray(bt2, np.float32), DC),
        gd=_chunk_vec(np.asarray(g_d, np.float32), DC),
        btd=_chunk_vec(np.asarray(bt_d, np.float32), DC),
    )
    in_maps = []
    for c in range(n_cores):
        rows = src[c * ns:(c + 1) * ns]
        shard = np.ascontiguousarray(rows.T).astype(np.float16)
        # estimated per-row top score (subtracted at the fp16 drain so the
        # candidate region lands near 0 where fp16 ulp is small)
        rb = -3.3 * np.sqrt((rows.astype(np.float64) ** 2).sum(1) + 256.0)
        rbv = np.ascontiguousarray(
            rb.reshape(ns // P, P).T.astype(np.float32))
        in_maps.append(dict(shared, srcT=shard, rowb=rbv))
    return in_maps


_NC_CACHE = {}


def kernel(**inputs):
    key = "full"
    if key not in _NC_CACHE:
        _NC_CACHE[key] = build_kernel()
    nc = _NC_CACHE[key]
    in_maps = prepare_inputs(**{k: np.asarray(v) for k, v in inputs.items()})
    res = run_bass_kernel_spmd(nc, in_maps, core_ids=list(range(N_CORES)))
    # per-core output is feature-major fp16 [d, ns]; transpose + cast
    return np.concatenate([r["out"].T.astype(np.float32)
                           for r in res.results], axis=0)
